# revision 39
# baseline (speedup 1.0000x reference)
"""Trainium2 Bass kernel for a cross-modal transformer block (attention + FFN).

Contract: kernel(**inputs) takes the FULL unsharded inputs (numpy, fp32) and
returns the FULL output [4, 2048, 512] fp32.

Sharding: 8 cores = data-parallel over batch (4) x query-sequence halves (2).
Each core computes K/V projections for its batch's full 2048-token sequence
(cheap duplication) so attention needs no collectives.

Device layout: everything feature-major ([features on partitions, tokens on
free]); the host pre-transposes and pre-casts inputs so the device does zero
transposes.
"""

import functools
import sys

import numpy as np

sys.path.insert(0, "/opt/trn_rl_repo")

import ml_dtypes  # noqa: E402

import concourse.bass as bass  # noqa: E402
import concourse.tile as tile  # noqa: E402
from concourse import bacc, mybir  # noqa: E402
from concourse.bass_utils import run_bass_kernel_spmd  # noqa: E402

BF16 = mybir.dt.bfloat16
F32 = mybir.dt.float32
AF = mybir.ActivationFunctionType
OP = mybir.AluOpType

B, S, D = 4, 2048, 512
H, DH = 8, 64
FF = 2048
P = 128
C = D // P  # 4 feature chunks
CF = FF // P  # 16 ffn chunks
TQ = S // 2  # 1024 query tokens per core
TK = S  # full key sequence per core
KC = TK // P  # 16 key chunks
NT = 512  # token tile (matmul free dim)
NQ = TQ // NT  # 2 query-token tiles
SCALE = 1.0 / np.sqrt(DH)  # 0.125
LN_EPS = 1e-5
NCORES = 8


def _emit(nc, t, es, tc):
    """Emit the per-core program. t: dict name -> DRAM AP."""
    # ---------------- pools ----------------
    pw = es.enter_context(tc.tile_pool(name="projw", bufs=1))
    wp = es.enter_context(tc.tile_pool(name="w", bufs=1))
    ap_ = es.enter_context(tc.tile_pool(name="acts", bufs=1))
    ptq = es.enter_context(tc.tile_pool(name="ptq", bufs=2))
    psS = es.enter_context(tc.tile_pool(name="psS", bufs=2, space="PSUM"))
    psC = es.enter_context(tc.tile_pool(name="psC", bufs=2, space="PSUM"))
    psE = es.enter_context(tc.tile_pool(name="psE", bufs=2, space="PSUM"))
    stream = es.enter_context(tc.tile_pool(name="stream", bufs=6))
    vpool = es.enter_context(tc.tile_pool(name="vpool", bufs=5))
    stage = es.enter_context(tc.tile_pool(name="stage", bufs=2))
    stage1 = es.enter_context(tc.tile_pool(name="stage1", bufs=1))
    chunk = es.enter_context(tc.tile_pool(name="chunk", bufs=2))
    chunk3 = es.enter_context(tc.tile_pool(name="chunk3", bufs=3))
    small = es.enter_context(tc.tile_pool(name="small", bufs=5))
    epool = es.enter_context(tc.tile_pool(name="e", bufs=4))
    hpool = es.enter_context(tc.tile_pool(name="h", bufs=1))

    def ld_w(pool, name, kchunks, n):
        w = pool.tile([P, kchunks, n], BF16, name=name + "_sb")
        src_ = t[name].rearrange("p (c o) -> p c o", c=kchunks)
        for ki in range(kchunks):
            nc.sync.dma_start(w[:, ki, :], src_[:, ki, :])
        return w

    # all small per-feature vectors arrive pre-shuffled in one [P, 48] pack
    ball = wp.tile([P, 48], F32, name="ball")
    nc.sync.dma_start(ball, t["ball"])
    bq, bk, bo, b2 = (ball[:, 4 * i : 4 * (i + 1)] for i in range(4))
    g1, be1, g2, be2 = (ball[:, 16 + 4 * i : 20 + 4 * i] for i in range(4))
    b1 = ball[:, 32:48]

    wk = ld_w(pw, "wk", C, D)
    kr0 = []
    for ki in range(C):
        r = stream.tile([P, 1024], BF16, tag="xr", name=f"kr_0_{ki}")
        nc.sync.dma_start(r[0:64], t["xkb"].rearrange("p (c q) -> p c q", c=C)[0:64, ki, 0:1024])
        nc.sync.dma_start(r[64:P], t["xkb"].rearrange("p (c q) -> p c q", c=C)[64:P, ki, 0:1024])
        kr0.append(r)
    wv = ld_w(pw, "wv", C, D)
    wq = ld_w(pw, "wq", C, D)
    bvb = pw.tile([P, D], F32)
    nc.sync.dma_start(bvb, t["bvb"])

    wo = ld_w(wp, "wo", C, D)
    w1d = t["w1"].rearrange("p (c o) -> p c o", c=C)
    w2d = t["w2"].rearrange("p (c o) -> p c o", c=CF)

    ones = wp.tile([P, 1], F32)
    nc.vector.memset(ones, 1.0)
    onesb = wp.tile([P, 1], BF16)
    nc.vector.memset(onesb, 1.0)
    epst = wp.tile([1, 1], F32)
    nc.vector.memset(epst, LN_EPS)

    # persistent activations (full key sequence); tags shared with the FFN
    # weights, which reuse these slots once attention is done
    kts = [ap_.tile([P, TK], BF16, tag=f"big1_{i}", name=f"kt_{i}") for i in range(C)]
    va = ap_.tile([P, KC, H, DH], BF16, tag="big2", name="va")  # V token-major

    xq32d = t["xq32"].rearrange("p (c q) -> p c q", c=C)
    xqbd = t["xqb"].rearrange("p (c q) -> p c q", c=C)
    xkb = t["xkb"].rearrange("p (c q) -> p c q", c=C)
    xvb = t["xvb"].rearrange("p (c q) -> p c q", c=C)
    out_d = t["out"].rearrange("(c p) q -> p c q", p=P)

    # ---------------- phase A: K/V/Q projections ----------------
    # inputs are loaded as [P, 1024] tiles (2KB DMA lines)
    for half in range(2):
        hs = slice(half * 1024, (half + 1) * 1024)
        if half == 0:
            kr = kr0
        else:
            kr = []
            for ki in range(C):
                r = stream.tile([P, 1024], BF16, tag="xr", name=f"kr_{half}_{ki}")
                nc.sync.dma_start(r[0:64], xkb[0:64, ki, hs])
                nc.sync.dma_start(r[64:P], xkb[64:P, ki, hs])
                kr.append(r)
        for tk in range(2):
            ts_ = slice(half * 1024 + tk * NT, half * 1024 + (tk + 1) * NT)
            tsl = slice(tk * NT, (tk + 1) * NT)
            for co in range(C):
                ps = psC.tile([P, NT], F32, tag="pc", name=f"kps_{half}_{tk}_{co}")
                for ki in range(C):
                    nc.tensor.matmul(
                        ps,
                        wk[:, ki, co * P : (co + 1) * P],
                        kr[ki][:, tsl],
                        start=(ki == 0),
                        stop=(ki == C - 1),
                    )
                nc.vector.tensor_scalar(
                    out=kts[co][:, ts_], in0=ps, scalar1=bk[:, co : co + 1],
                    scalar2=None, op0=OP.add,
                )

    # V projection, token-major out: V = Xv @ Wv  (lhsT = Xv.T chunk)
    for half in range(2):
        hs = slice(half * 1024, (half + 1) * 1024)
        vr = []
        for ki in range(C):
            r = stream.tile([P, 1024], BF16, tag="xr", name=f"vr_{half}_{ki}")
            nc.sync.dma_start(r[0:64], xvb[0:64, ki, hs])
            nc.sync.dma_start(r[64:P], xvb[64:P, ki, hs])
            vr.append(r)
        for tm8 in range(8):
            tm = half * 8 + tm8
            msl = slice(tm8 * P, (tm8 + 1) * P)
            ps = psC.tile([P, NT], F32, tag="pc", name=f"vps_{tm}")
            for ki in range(C):
                nc.tensor.matmul(
                    ps, vr[ki][:, msl], wv[:, ki, :],
                    start=(ki == 0), stop=(ki == C - 1),
                )
            nc.vector.tensor_tensor(
                out=va[:, tm, :, :],
                in0=ps.rearrange("p (h d) -> p h d", h=H),
                in1=bvb.rearrange("p (h d) -> p h d", h=H),
                op=OP.add,
            )

    # Q projections for both token tiles (so attention for either tile is
    # never blocked on projection work)
    qr = []
    for ki in range(C):
        r = stream.tile([P, TQ], BF16, tag="xr", name=f"qr_{ki}")
        nc.sync.dma_start(r[0:64], xqbd[0:64, ki, :])
        nc.sync.dma_start(r[64:P], xqbd[64:P, ki, :])
        qr.append(r)
    qts = []
    for tq in range(NQ):
        tsl = slice(tq * NT, (tq + 1) * NT)
        qt = ptq.tile([P, C, NT], BF16, tag="qt", name=f"qt_{tq}")
        for co in range(C):
            ps = psC.tile([P, NT], F32, tag="pc", name=f"qps_{tq}_{co}")
            for ki in range(C):
                nc.tensor.matmul(
                    ps,
                    wq[:, ki, co * P : (co + 1) * P],
                    qr[ki][:, tsl],
                    start=(ki == 0),
                    stop=(ki == C - 1),
                )
            nc.vector.tensor_scalar(
                out=qt[:, co, :], in0=ps, scalar1=bq[:, co : co + 1],
                scalar2=None, op0=OP.add,
            )
        qts.append(qt)

    # ---------------- phase B: attention (both token tiles) ----------------
    ctxs = []
    for tq in range(NQ):
        qt = qts[tq]
        ctx = ptq.tile([P, C, NT], BF16, tag="ctx", name=f"ctx_{tq}")
        for hp in range(H // 2):  # head pairs sharing a 128-partition chunk
            pc = [
                psC.tile([P, NT], F32, tag="pc", name=f"pc_{tq}_{hp}_{j}")
                for j in range(2)
            ]
            # software-pipelined: scores(kc)+exp(kc) emitted before ctx(kc-1)
            e2s = [None] * KC
            for kc in range(KC + 1):
                if kc < KC:
                    ksl = slice(kc * P, (kc + 1) * P)
                    ps2 = psS.tile(
                        [P, 2, NT], F32, tag="ps2", name=f"sps_{tq}_{hp}_{kc}"
                    )
                    e2 = epool.tile(
                        [P, 2, NT], BF16, tag="e", name=f"e_{tq}_{hp}_{kc}"
                    )
                    for j in range(2):  # head 2*hp + j at partition offset 64*j
                        rows = slice(j * DH, (j + 1) * DH)
                        # scores.T chunk = K_h @ Q_h.T
                        nc.tensor.matmul(
                            ps2[:, j, :], kt[rows, hp, ksl], qt[rows, hp, :],
                            start=True, stop=True,
                        )
                    nc.scalar.activation(e2, ps2, AF.Exp, scale=SCALE)
                    e2s[kc] = e2
                if kc >= 1:
                    for j in range(2):
                        # ctx.T (+ sumexp row 64): lhsT = [V_h | 1], rhs = E.T
                        nc.tensor.matmul(
                            pc[j][0 : DH + 1, :],
                            va[:, kc - 1, 2 * hp + j, :],
                            e2s[kc - 1][:, j, :],
                            start=(kc - 1 == 0),
                            stop=(kc - 1 == KC - 1),
                        )
            for j in range(2):
                # fast copies release the PSUM accumulator; approx reciprocal
                # (~4e-6 rel err, plenty for a softmax denominator) keeps the
                # DVE FIFO clear
                se = small.tile([1, NT], F32, tag="sm", name=f"se_{tq}_{hp}_{j}")
                nc.vector.tensor_copy(out=se, in_=pc[j][DH : DH + 1, :])
                cf = chunk.tile([DH, NT], F32, tag="cf", name=f"cf_{tq}_{hp}_{j}")
                nc.vector.tensor_copy(out=cf, in_=pc[j][0:DH, :])
                rc = small.tile([1, NT], F32, tag="sm", name=f"rc_{tq}_{hp}_{j}")
                nc.vector.reciprocal_approx_fast(out=rc, in_=se)
                db = chunk.tile([DH, NT], F32, tag="db", name=f"db_{tq}_{hp}_{j}")
                nc.gpsimd.partition_broadcast(db, rc)
                nc.vector.tensor_tensor(
                    out=ctx[j * DH : (j + 1) * DH, hp, :],
                    in0=cf,
                    in1=db,
                    op=OP.mult,
                )
        ctxs.append(ctx)

    # FFN weights reuse the kt/va slots (attention is done with them);
    # 4 DMAs each so the loads spread across queues and overlap Oproj/LN1
    w1s = ap_.tile([P, C, FF], BF16, tag="big1", name="w1s")
    for ki in range(C):
        nc.sync.dma_start(w1s[:, ki, :], w1d[:, ki, :])
    w2s = ap_.tile([P, CF, D], BF16, tag="big2", name="w2s")
    for kq in range(4):
        nc.sync.dma_start(w2s[:, 4 * kq : 4 * kq + 4, :], w2d[:, 4 * kq : 4 * kq + 4, :])

    # ---------------- phase C: tails, interleaved across token tiles ------
    def layernorm(resid, g, be, out_write, tag, out_write_co=None):
        """resid: [P, C, NT] f32 tile. out_write(co, t2_f32_tile, be_col)."""
        lnp = psC.tile([P, NT], F32, tag="pc", name=f"lnp_{tag}")
        for co in range(C):
            nc.tensor.matmul(lnp[0:1, :], ones, resid[:, co, :], start=(co == 0),
                             stop=(co == C - 1), skip_group_check=True)
        s4 = stage1.tile([P, C, NT], F32, tag="sq", name=f"sq_{tag}")
        nc.vector.tensor_mul(s4, resid, resid)
        for co in range(C):
            nc.tensor.matmul(lnp[64:65, :], ones, s4[:, co, :], start=(co == 0),
                             stop=(co == C - 1), tile_position=(0, 64),
                             skip_group_check=True)
        mean = small.tile([1, NT], F32, tag="sm", name=f"mean_{tag}")
        nc.vector.tensor_scalar_mul(mean, lnp[0:1, :], 1.0 / D)
        msq = small.tile([1, NT], F32, tag="sm", name=f"msq_{tag}")
        nc.vector.tensor_scalar_mul(msq, lnp[64:65, :], 1.0 / D)
        m2 = small.tile([1, NT], F32, tag="sm", name=f"m2_{tag}")
        nc.vector.tensor_mul(m2, mean, mean)
        var = small.tile([1, NT], F32, tag="sm", name=f"var_{tag}")
        nc.vector.tensor_tensor(out=var, in0=msq, in1=m2, op=OP.subtract)
        # rstd = exp(-0.5 * ln(var + eps)) -- stays in the Exp/Ln ACT table set
        lnv = small.tile([1, NT], F32, tag="sm", name=f"lnv_{tag}")
        nc.scalar.activation(lnv, var, AF.Ln, bias=epst)
        rstd = small.tile([1, NT], F32, tag="sm", name=f"rstd_{tag}")
        nc.scalar.activation(rstd, lnv, AF.Exp, scale=-0.5)
        meanb = chunk.tile([P, NT], F32, tag="bc", name=f"meanb_{tag}")
        nc.gpsimd.partition_broadcast(meanb, mean)
        rstdb = chunk.tile([P, NT], F32, tag="bc", name=f"rstdb_{tag}")
        nc.gpsimd.partition_broadcast(rstdb, rstd)
        tt = stage.tile([P, C, NT], F32, tag="lnt", name=f"tt_{tag}")
        nc.vector.tensor_tensor(
            out=tt, in0=resid,
            in1=meanb[:, None, :].to_broadcast((P, C, NT)), op=OP.subtract,
        )
        nc.vector.tensor_tensor(
            out=tt, in0=tt,
            in1=rstdb[:, None, :].to_broadcast((P, C, NT)), op=OP.mult,
        )
        nc.vector.tensor_tensor(
            out=tt, in0=tt,
            in1=g[:, :, None].to_broadcast((P, C, NT)), op=OP.mult,
        )
        out_write(tt, be)

    # O projection + residual (query + attn_out)
    resids = []
    for tq in range(NQ):
        ts_ = slice(tq * NT, (tq + 1) * NT)
        ctx = ctxs[tq]
        resid = stage.tile([P, C, NT], F32, tag="resid", name=f"resid_{tq}")
        for co in range(C):
            xqc = chunk.tile([P, NT], F32, tag="xqc", name=f"xqc_{tq}_{co}")
            nc.sync.dma_start(xqc, xq32d[:, co, ts_])
            ps = psC.tile([P, NT], F32, tag="pc", name=f"ops_{tq}_{co}")
            for ki in range(C):
                nc.tensor.matmul(
                    ps,
                    wo[:, ki, co * P : (co + 1) * P],
                    ctx[:, ki, :],
                    start=(ki == 0),
                    stop=(ki == C - 1),
                )
            nc.vector.scalar_tensor_tensor(
                out=resid[:, co, :],
                in0=ps,
                scalar=bo[:, co : co + 1],
                in1=xqc,
                op0=OP.add,
                op1=OP.add,
            )
        resids.append(resid)

    # LN1 for both tiles first (their chains overlap each other and the
    # Oproj tail), then per-tq FFN1/FFN2/LN2 (each LN2 chain overlaps the
    # next tile's FFN matmuls)
    ln1fs, ln1bs = [], []
    for tq in range(NQ):
        ln1f = stage.tile([P, C, NT], F32, tag="ln1f", name=f"ln1f_{tq}")
        ln1b = ptq.tile([P, C, NT], BF16, tag="ln1b", name=f"ln1b_{tq}")

        def write_ln1(tt, be, ln1f=ln1f, ln1b=ln1b):
            nc.vector.tensor_tensor(
                out=ln1f, in0=tt,
                in1=be[:, :, None].to_broadcast((P, C, NT)), op=OP.add,
            )
            nc.vector.tensor_copy(out=ln1b, in_=ln1f)

        layernorm(resids[tq], g1, be1, write_ln1, f"l1_{tq}")
        ln1fs.append(ln1f)
        ln1bs.append(ln1b)

    for tq in range(NQ):
        ts_ = slice(tq * NT, (tq + 1) * NT)
        hb = hpool.tile([P, CF, NT], BF16, tag="h", name=f"h_{tq}")
        for fo in range(CF):
            ps = psC.tile([P, NT], F32, tag="pc", name=f"fps_{tq}_{fo}")
            for ki in range(C):
                nc.tensor.matmul(
                    ps,
                    w1s[:, ki, fo * P : (fo + 1) * P],
                    ln1bs[tq][:, ki, :],
                    start=(ki == 0),
                    stop=(ki == C - 1),
                )
            nc.scalar.activation(hb[:, fo, :], ps, AF.Gelu, bias=b1[:, fo : fo + 1])

        resid2 = stage1.tile([P, C, NT], F32, tag="resid2", name=f"resid2_{tq}")
        for co in range(C):
            ps = psC.tile([P, NT], F32, tag="pc", name=f"gps_{tq}_{co}")
            for ki in range(CF):
                nc.tensor.matmul(
                    ps,
                    w2s[:, ki, co * P : (co + 1) * P],
                    hb[:, ki, :],
                    start=(ki == 0),
                    stop=(ki == CF - 1),
                )
            nc.vector.scalar_tensor_tensor(
                out=resid2[:, co, :],
                in0=ps,
                scalar=b2[:, co : co + 1],
                in1=ln1fs[tq][:, co, :],
                op0=OP.add,
                op1=OP.add,
            )

        def write_out(tt, be, ts_=ts_, tq=tq):
            oc = stage.tile([P, C, NT], F32, tag="lnt", name=f"oc_{tq}")
            nc.vector.tensor_tensor(
                out=oc, in0=tt,
                in1=be[:, :, None].to_broadcast((P, C, NT)), op=OP.add,
            )
            for co in range(C):
                nc.sync.dma_start(out_d[:, co, ts_], oc[:, co, :])

        layernorm(resid2, g2, be2, write_out, f"l2_{tq}")


# revision 40
# speedup vs baseline: 1.7393x; 1.7393x over previous
"""Trainium2 Bass kernel for a cross-modal transformer block (attention + FFN).

Contract: kernel(**inputs) takes the FULL unsharded inputs (numpy, fp32) and
returns the FULL output [4, 2048, 512] fp32.

Sharding: 8 cores = data-parallel over batch (4) x query-sequence halves (2).
Each core computes K/V projections for its batch's full 2048-token sequence
(cheap duplication) so attention needs no collectives.

Device layout: everything feature-major ([features on partitions, tokens on
free]); the host pre-transposes and pre-casts inputs so the device does zero
transposes.
"""

import functools
import sys

import numpy as np

sys.path.insert(0, "/opt/trn_rl_repo")

import ml_dtypes  # noqa: E402

import concourse.bass as bass  # noqa: E402
import concourse.tile as tile  # noqa: E402
from concourse import bacc, mybir  # noqa: E402
from concourse.bass_utils import run_bass_kernel_spmd  # noqa: E402

BF16 = mybir.dt.bfloat16
F32 = mybir.dt.float32
AF = mybir.ActivationFunctionType
OP = mybir.AluOpType

B, S, D = 4, 2048, 512
H, DH = 8, 64
FF = 2048
P = 128
C = D // P  # 4 feature chunks
CF = FF // P  # 16 ffn chunks
TQ = S // 2  # 1024 query tokens per core
TK = S  # full key sequence per core
KC = TK // P  # 16 key chunks
NT = 512  # token tile (matmul free dim)
NQ = TQ // NT  # 2 query-token tiles
SCALE = 1.0 / np.sqrt(DH)  # 0.125
LN_EPS = 1e-5
NCORES = 8


def _emit(nc, t, es, tc):
    """Emit the per-core program. t: dict name -> DRAM AP."""
    # ---------------- pools ----------------
    pw = es.enter_context(tc.tile_pool(name="projw", bufs=1))
    wp = es.enter_context(tc.tile_pool(name="w", bufs=1))
    ap_ = es.enter_context(tc.tile_pool(name="acts", bufs=1))
    ptq = es.enter_context(tc.tile_pool(name="ptq", bufs=2))
    psS = es.enter_context(tc.tile_pool(name="psS", bufs=2, space="PSUM"))
    psC = es.enter_context(tc.tile_pool(name="psC", bufs=3, space="PSUM"))
    psE = es.enter_context(tc.tile_pool(name="psE", bufs=1, space="PSUM"))
    stream = es.enter_context(tc.tile_pool(name="stream", bufs=6))
    vpool = es.enter_context(tc.tile_pool(name="vpool", bufs=5))
    stage = es.enter_context(tc.tile_pool(name="stage", bufs=2))
    stage1 = es.enter_context(tc.tile_pool(name="stage1", bufs=1))
    chunk = es.enter_context(tc.tile_pool(name="chunk", bufs=2))
    chunk3 = es.enter_context(tc.tile_pool(name="chunk3", bufs=3))
    small = es.enter_context(tc.tile_pool(name="small", bufs=6))
    epool = es.enter_context(tc.tile_pool(name="e", bufs=6))
    hpool = es.enter_context(tc.tile_pool(name="h", bufs=1))

    def ld_w(pool, name, kchunks, n):
        w = pool.tile([P, kchunks, n], BF16, name=name + "_sb")
        src_ = t[name].rearrange("p (c o) -> p c o", c=kchunks)
        for ki in range(kchunks):
            nc.sync.dma_start(w[:, ki, :], src_[:, ki, :])
        return w

    # all small per-feature vectors arrive pre-shuffled in one [P, 48] pack
    ball = wp.tile([P, 48], F32, name="ball")
    nc.sync.dma_start(ball, t["ball"])
    bq, bk, bo, b2 = (ball[:, 4 * i : 4 * (i + 1)] for i in range(4))
    g1, be1, g2, be2 = (ball[:, 16 + 4 * i : 20 + 4 * i] for i in range(4))
    b1 = ball[:, 32:48]

    wk = ld_w(pw, "wk", C, D)
    kr0 = []
    for ki in range(C):
        r = stream.tile([P, 1024], BF16, tag="xr", name=f"kr_0_{ki}")
        nc.sync.dma_start(r[0:64], t["xkb"].rearrange("p (c q) -> p c q", c=C)[0:64, ki, 0:1024])
        nc.sync.dma_start(r[64:P], t["xkb"].rearrange("p (c q) -> p c q", c=C)[64:P, ki, 0:1024])
        kr0.append(r)
    wv = ld_w(pw, "wv", C, D)
    wq = ld_w(pw, "wq", C, D)
    bvb = pw.tile([P, D], F32)
    nc.sync.dma_start(bvb, t["bvb"])

    wo = ld_w(wp, "wo", C, D)
    w1d = t["w1"].rearrange("p (c o) -> p c o", c=C)
    w2d = t["w2"].rearrange("p (c o) -> p c o", c=CF)

    ones = wp.tile([P, 1], F32)
    nc.vector.memset(ones, 1.0)
    onesb = wp.tile([P, 1], BF16)
    nc.vector.memset(onesb, 1.0)
    epst = wp.tile([1, 1], F32)
    nc.vector.memset(epst, LN_EPS)

    # persistent activations (full key sequence); tags shared with the FFN
    # weights, which reuse these slots once attention is done
    kts = [ap_.tile([P, TK], BF16, tag=f"big1_{i}", name=f"kt_{i}") for i in range(C)]
    va = ap_.tile([P, KC, H, DH], BF16, tag="big2", name="va")  # V token-major

    xq32d = t["xq32"].rearrange("p (c q) -> p c q", c=C)
    xqbd = t["xqb"].rearrange("p (c q) -> p c q", c=C)
    xkb = t["xkb"].rearrange("p (c q) -> p c q", c=C)
    xvb = t["xvb"].rearrange("p (c q) -> p c q", c=C)
    out_d = t["out"].rearrange("(c p) q -> p c q", p=P)

    # ---------------- phase A: K/V/Q projections ----------------
    # inputs are loaded as [P, 1024] tiles (2KB DMA lines)
    for half in range(2):
        hs = slice(half * 1024, (half + 1) * 1024)
        if half == 0:
            kr = kr0
        else:
            kr = []
            for ki in range(C):
                r = stream.tile([P, 1024], BF16, tag="xr", name=f"kr_{half}_{ki}")
                nc.sync.dma_start(r[0:64], xkb[0:64, ki, hs])
                nc.sync.dma_start(r[64:P], xkb[64:P, ki, hs])
                kr.append(r)
        for tk in range(2):
            ts_ = slice(half * 1024 + tk * NT, half * 1024 + (tk + 1) * NT)
            tsl = slice(tk * NT, (tk + 1) * NT)
            for co in range(C):
                ps = psC.tile([P, NT], F32, tag="pc", name=f"kps_{half}_{tk}_{co}")
                for ki in range(C):
                    nc.tensor.matmul(
                        ps,
                        wk[:, ki, co * P : (co + 1) * P],
                        kr[ki][:, tsl],
                        start=(ki == 0),
                        stop=(ki == C - 1),
                    )
                nc.vector.tensor_scalar(
                    out=kts[co][:, ts_], in0=ps, scalar1=bk[:, co : co + 1],
                    scalar2=None, op0=OP.add,
                )

    # V projection, token-major out: V = Xv @ Wv  (lhsT = Xv.T chunk)
    for half in range(2):
        hs = slice(half * 1024, (half + 1) * 1024)
        vr = []
        for ki in range(C):
            r = stream.tile([P, 1024], BF16, tag="xr", name=f"vr_{half}_{ki}")
            nc.sync.dma_start(r[0:64], xvb[0:64, ki, hs])
            nc.sync.dma_start(r[64:P], xvb[64:P, ki, hs])
            vr.append(r)
        for tm8 in range(8):
            tm = half * 8 + tm8
            msl = slice(tm8 * P, (tm8 + 1) * P)
            ps = psC.tile([P, NT], F32, tag="pc", name=f"vps_{tm}")
            for ki in range(C):
                nc.tensor.matmul(
                    ps, vr[ki][:, msl], wv[:, ki, :],
                    start=(ki == 0), stop=(ki == C - 1),
                )
            nc.vector.tensor_tensor(
                out=va[:, tm, :, :],
                in0=ps.rearrange("p (h d) -> p h d", h=H),
                in1=bvb.rearrange("p (h d) -> p h d", h=H),
                op=OP.add,
            )

    # Q projections for both token tiles (so attention for either tile is
    # never blocked on projection work)
    qr = []
    for ki in range(C):
        r = stream.tile([P, TQ], BF16, tag="xr", name=f"qr_{ki}")
        nc.sync.dma_start(r[0:64], xqbd[0:64, ki, :])
        nc.sync.dma_start(r[64:P], xqbd[64:P, ki, :])
        qr.append(r)
    qts = []
    for tq in range(NQ):
        tsl = slice(tq * NT, (tq + 1) * NT)
        qt = ptq.tile([P, C, NT], BF16, tag="qt", name=f"qt_{tq}")
        for co in range(C):
            ps = psC.tile([P, NT], F32, tag="pc", name=f"qps_{tq}_{co}")
            for ki in range(C):
                nc.tensor.matmul(
                    ps,
                    wq[:, ki, co * P : (co + 1) * P],
                    qr[ki][:, tsl],
                    start=(ki == 0),
                    stop=(ki == C - 1),
                )
            nc.vector.tensor_scalar(
                out=qt[:, co, :], in0=ps, scalar1=bq[:, co : co + 1],
                scalar2=None, op0=OP.add,
            )
        qts.append(qt)

    # ---------------- phase B: attention (both token tiles) ----------------
    ctxs = []
    for tq in range(NQ):
        qt = qts[tq]
        ctx = ptq.tile([P, C, NT], BF16, tag="ctx", name=f"ctx_{tq}")
        for hp in range(H // 2):  # head pairs sharing a 128-partition chunk
            pc = [
                psC.tile([P, NT], F32, tag="pc", name=f"pc_{tq}_{hp}_{j}")
                for j in range(2)
            ]
            # software-pipelined: scores(kc)+exp(kc) emitted before ctx(kc-1)
            e2s = [None] * KC
            for kc in range(KC + 1):
                if kc < KC:
                    ksl = slice(kc * P, (kc + 1) * P)
                    ps2 = psS.tile(
                        [P, 2, NT], F32, tag="ps2", name=f"sps_{tq}_{hp}_{kc}"
                    )
                    e2 = epool.tile(
                        [P, 2, NT], BF16, tag="e", name=f"e_{tq}_{hp}_{kc}"
                    )
                    for j in range(2):  # head 2*hp + j at partition offset 64*j
                        rows = slice(j * DH, (j + 1) * DH)
                        # scores.T chunk = K_h @ Q_h.T
                        nc.tensor.matmul(
                            ps2[:, j, :], kt[rows, hp, ksl], qt[rows, hp, :],
                            start=True, stop=True,
                        )
                    nc.scalar.activation(e2, ps2, AF.Exp, scale=SCALE)
                    e2s[kc] = e2
                if kc >= 1:
                    for j in range(2):
                        # ctx.T (+ sumexp row 64): lhsT = [V_h | 1], rhs = E.T
                        nc.tensor.matmul(
                            pc[j][0 : DH + 1, :],
                            va[:, kc - 1, 2 * hp + j, :],
                            e2s[kc - 1][:, j, :],
                            start=(kc - 1 == 0),
                            stop=(kc - 1 == KC - 1),
                        )
            for j in range(2):
                # fast copies release the PSUM accumulator; approx reciprocal
                # (~4e-6 rel err, plenty for a softmax denominator) keeps the
                # DVE FIFO clear
                se = small.tile([1, NT], F32, tag="sm", name=f"se_{tq}_{hp}_{j}")
                nc.vector.tensor_copy(out=se, in_=pc[j][DH : DH + 1, :])
                cf = chunk.tile([DH, NT], F32, tag="cf", name=f"cf_{tq}_{hp}_{j}")
                nc.vector.tensor_copy(out=cf, in_=pc[j][0:DH, :])
                rc = small.tile([1, NT], F32, tag="sm", name=f"rc_{tq}_{hp}_{j}")
                nc.vector.reciprocal_approx_fast(out=rc, in_=se)
                db = chunk.tile([DH, NT], F32, tag="db", name=f"db_{tq}_{hp}_{j}")
                nc.gpsimd.partition_broadcast(db, rc)
                nc.vector.tensor_tensor(
                    out=ctx[j * DH : (j + 1) * DH, hp, :],
                    in0=cf,
                    in1=db,
                    op=OP.mult,
                )
        ctxs.append(ctx)

    # FFN weights reuse the kt/va slots (attention is done with them);
    # 4 DMAs each so the loads spread across queues and overlap Oproj/LN1
    w1s = ap_.tile([P, C, FF], BF16, tag="big1", name="w1s")
    for ki in range(C):
        nc.sync.dma_start(w1s[:, ki, :], w1d[:, ki, :])
    w2s = ap_.tile([P, CF, D], BF16, tag="big2", name="w2s")
    for kq in range(4):
        nc.sync.dma_start(w2s[:, 4 * kq : 4 * kq + 4, :], w2d[:, 4 * kq : 4 * kq + 4, :])

    # ---------------- phase C: tails, interleaved across token tiles ------
    def layernorm(resid, g, be, out_write, tag, out_write_co=None):
        """resid: [P, C, NT] f32 tile. out_write(co, t2_f32_tile, be_col)."""
        lnp = psC.tile([P, NT], F32, tag="pc", name=f"lnp_{tag}")
        for co in range(C):
            nc.tensor.matmul(lnp[0:1, :], ones, resid[:, co, :], start=(co == 0),
                             stop=(co == C - 1), skip_group_check=True)
        s4 = stage1.tile([P, C, NT], F32, tag="sq", name=f"sq_{tag}")
        nc.vector.tensor_mul(s4, resid, resid)
        for co in range(C):
            nc.tensor.matmul(lnp[64:65, :], ones, s4[:, co, :], start=(co == 0),
                             stop=(co == C - 1), tile_position=(0, 64),
                             skip_group_check=True)
        mean = small.tile([1, NT], F32, tag="sm", name=f"mean_{tag}")
        nc.vector.tensor_scalar_mul(mean, lnp[0:1, :], 1.0 / D)
        msq = small.tile([1, NT], F32, tag="sm", name=f"msq_{tag}")
        nc.vector.tensor_scalar_mul(msq, lnp[64:65, :], 1.0 / D)
        m2 = small.tile([1, NT], F32, tag="sm", name=f"m2_{tag}")
        nc.vector.tensor_mul(m2, mean, mean)
        var = small.tile([1, NT], F32, tag="sm", name=f"var_{tag}")
        nc.vector.tensor_tensor(out=var, in0=msq, in1=m2, op=OP.subtract)
        # rstd = exp(-0.5 * ln(var + eps)) -- stays in the Exp/Ln ACT table set
        lnv = small.tile([1, NT], F32, tag="sm", name=f"lnv_{tag}")
        nc.scalar.activation(lnv, var, AF.Ln, bias=epst)
        rstd = small.tile([1, NT], F32, tag="sm", name=f"rstd_{tag}")
        nc.scalar.activation(rstd, lnv, AF.Exp, scale=-0.5)
        meanb = chunk.tile([P, NT], F32, tag="bc", name=f"meanb_{tag}")
        nc.gpsimd.partition_broadcast(meanb, mean)
        rstdb = chunk.tile([P, NT], F32, tag="bc", name=f"rstdb_{tag}")
        nc.gpsimd.partition_broadcast(rstdb, rstd)
        tt = stage.tile([P, C, NT], F32, tag="lnt", name=f"tt_{tag}")
        nc.vector.tensor_tensor(
            out=tt, in0=resid,
            in1=meanb[:, None, :].to_broadcast((P, C, NT)), op=OP.subtract,
        )
        nc.vector.tensor_tensor(
            out=tt, in0=tt,
            in1=rstdb[:, None, :].to_broadcast((P, C, NT)), op=OP.mult,
        )
        nc.vector.tensor_tensor(
            out=tt, in0=tt,
            in1=g[:, :, None].to_broadcast((P, C, NT)), op=OP.mult,
        )
        out_write(tt, be)

    # O projection + residual (query + attn_out)
    resids = []
    for tq in range(NQ):
        ts_ = slice(tq * NT, (tq + 1) * NT)
        ctx = ctxs[tq]
        resid = stage.tile([P, C, NT], F32, tag="resid", name=f"resid_{tq}")
        for co in range(C):
            xqc = chunk.tile([P, NT], F32, tag="xqc", name=f"xqc_{tq}_{co}")
            nc.sync.dma_start(xqc, xq32d[:, co, ts_])
            ps = psC.tile([P, NT], F32, tag="pc", name=f"ops_{tq}_{co}")
            for ki in range(C):
                nc.tensor.matmul(
                    ps,
                    wo[:, ki, co * P : (co + 1) * P],
                    ctx[:, ki, :],
                    start=(ki == 0),
                    stop=(ki == C - 1),
                )
            nc.vector.scalar_tensor_tensor(
                out=resid[:, co, :],
                in0=ps,
                scalar=bo[:, co : co + 1],
                in1=xqc,
                op0=OP.add,
                op1=OP.add,
            )
        resids.append(resid)

    # LN1 for both tiles first (their chains overlap each other and the
    # Oproj tail), then per-tq FFN1/FFN2/LN2 (each LN2 chain overlaps the
    # next tile's FFN matmuls)
    ln1fs, ln1bs = [], []
    for tq in range(NQ):
        ln1f = stage.tile([P, C, NT], F32, tag="ln1f", name=f"ln1f_{tq}")
        ln1b = ptq.tile([P, C, NT], BF16, tag="ln1b", name=f"ln1b_{tq}")

        def write_ln1(tt, be, ln1f=ln1f, ln1b=ln1b):
            nc.vector.tensor_tensor(
                out=ln1f, in0=tt,
                in1=be[:, :, None].to_broadcast((P, C, NT)), op=OP.add,
            )
            nc.vector.tensor_copy(out=ln1b, in_=ln1f)

        layernorm(resids[tq], g1, be1, write_ln1, f"l1_{tq}")
        ln1fs.append(ln1f)
        ln1bs.append(ln1b)

    for tq in range(NQ):
        ts_ = slice(tq * NT, (tq + 1) * NT)
        hb = hpool.tile([P, CF, NT], BF16, tag="h", name=f"h_{tq}")
        for fo in range(CF):
            ps = psC.tile([P, NT], F32, tag="pc", name=f"fps_{tq}_{fo}")
            for ki in range(C):
                nc.tensor.matmul(
                    ps,
                    w1s[:, ki, fo * P : (fo + 1) * P],
                    ln1bs[tq][:, ki, :],
                    start=(ki == 0),
                    stop=(ki == C - 1),
                )
            nc.scalar.activation(hb[:, fo, :], ps, AF.Gelu, bias=b1[:, fo : fo + 1])

        resid2 = stage1.tile([P, C, NT], F32, tag="resid2", name=f"resid2_{tq}")
        for co in range(C):
            ps = psC.tile([P, NT], F32, tag="pc", name=f"gps_{tq}_{co}")
            for ki in range(CF):
                nc.tensor.matmul(
                    ps,
                    w2s[:, ki, co * P : (co + 1) * P],
                    hb[:, ki, :],
                    start=(ki == 0),
                    stop=(ki == CF - 1),
                )
            nc.vector.scalar_tensor_tensor(
                out=resid2[:, co, :],
                in0=ps,
                scalar=b2[:, co : co + 1],
                in1=ln1fs[tq][:, co, :],
                op0=OP.add,
                op1=OP.add,
            )

        def write_out(tt, be, ts_=ts_, tq=tq):
            oc = stage.tile([P, C, NT], F32, tag="lnt", name=f"oc_{tq}")
            nc.vector.tensor_tensor(
                out=oc, in0=tt,
                in1=be[:, :, None].to_broadcast((P, C, NT)), op=OP.add,
            )
            for co in range(C):
                nc.sync.dma_start(out_d[:, co, ts_], oc[:, co, :])

        layernorm(resid2, g2, be2, write_out, f"l2_{tq}")
